# revision 1
# baseline (speedup 1.0000x reference)
"""Trainium2 Bass kernel for the autoregressive LSTM decoder problem.

v3: baseline structure with i,f,o gate matmuls switched to fp8e4 DoubleRow
(2x PE throughput; contraction 256/instr so 2 matmuls instead of 4 per
gate tile); g gate stays bf16 (it dominates fp8 error: all-fp8 fails at
2.7e-2, fp8-ifo measures 3.6e-3). h additionally mirrored into fp8
interleaved tiles by one extra DVE copy per (k,n).

Full-input contract: kernel(**inputs) takes the unsharded numpy inputs
(B=8192, D=512, K=24) and returns (out1, out2), each [B, K] float32.

Strategy (data-parallel over 8 NeuronCores, B/8 = 1024 batch per core):
  * All state is kept transposed on-chip: h,c as [D, B_shard] so the
    per-step gate matmul z^T = Wu^T @ h^T + Wx^T x^T lands in PSUM already
    gate-major; no transposes anywhere in the decode loop.
  * Matmul operands are bf16 (fp32 PSUM accumulation); all elementwise
    math is fp32. The LSTM's saturating gates keep the bf16 matmul noise
    ~2e-4 (y1) / 3e-3 (y2) scale-relative at the outputs.
  * The rank-1 x@Wx term closes each 5-matmul PSUM accumulation group as a
    K=1 matmul; the four closers of a k-group go to four distinct 32-row PE
    tile_positions back-to-back so they run concurrently on the PE array.
  * ScalarE applies sigmoid/tanh directly PSUM->SBUF (fused evacuation);
    VectorE does the c/h updates; y2's elu is deferred and applied once,
    batched [K, B_shard], after the step loop.
"""

import sys

import numpy as np

for _p in ("/opt/trn_rl_repo", "/root/.axon_site/_ro/trn_rl_repo"):
    if _p not in sys.path:
        sys.path.append(_p)

import concourse.bass as bass
import concourse.mybir as mybir
from concourse.tile import TileContext
from concourse.vector_clock import ScopedClock

F32 = mybir.dt.float32
BF16 = mybir.dt.bfloat16
FP8 = mybir.dt.float8e4
DR = mybir.MatmulPerfMode.DoubleRow
AF = mybir.ActivationFunctionType

D = 512
B = 1024          # batch per core
NCORES = 8
K = 24
G = 4 * D         # 2048 gate rows
KT = D // 128     # 4 k tiles
MT = G // 128     # 16 gate row tiles
NT = B // 512     # 2 batch chunks
N = 512

_MAX_WAITS_PER_DRAIN = 1


def _split_waits(nc):
    """The walrus build in this container accepts at most one semaphore wait
    per instruction. Rebuild every basic block, hoisting all-but-one wait of
    any overloaded instruction onto same-engine InstEventSemaphore
    instructions inserted immediately before it — the engine blocks at the
    same program point for the same conditions, so this is
    semantics-preserving."""
    n_new = 0
    for f in nc.m.functions:
        for blk in f.blocks:
            insts = list(blk.instructions)
            out = []
            changed = False
            for inst in insts:
                si = inst.sync_info
                waits = list(si.on_wait) if si is not None else []
                if len(waits) > 1:
                    changed = True
                    excess, keep = waits[:-1], waits[-1:]
                    for w in excess:
                        ev = mybir.InstEventSemaphore(
                            name=f"splitw-{n_new}", ins=[], outs=[],
                            engine=inst.engine,
                        )
                        ev.sync_info = mybir.SyncInfo(on_wait=[w], on_update=[])
                        nc.register_instruction(ev, overwrite=True)
                        out.append(ev)
                        n_new += 1
                    inst.sync_info = mybir.SyncInfo(
                        on_wait=keep, on_update=list(si.on_update)
                    )
                out.append(inst)
            if changed:
                blk.instructions = out
    return n_new


class SplitDrainTileContext(TileContext):
    """The walrus build in this container rejects Drain (CTRL_NO)
    instructions carrying more than ~2 sync waits; split the tail drain's
    waits across a chain of Drain instructions, one wait each."""

    def _drain_and_barrier(self, tick_clock, wait_clock):
        nc = self.nc
        drain_inst = nc.sync.drain()
        wait_clock.add_sem_waits(
            drain_inst.ins, ScopedClock({None: tick_clock.global_clock})
        )
        si = drain_inst.ins.sync_info
        waits = list(si.on_wait) if si is not None else []
        if len(waits) > _MAX_WAITS_PER_DRAIN:
            drain_inst.ins.sync_info = mybir.SyncInfo(
                on_wait=waits[:_MAX_WAITS_PER_DRAIN], on_update=[]
            )
            for i in range(_MAX_WAITS_PER_DRAIN, len(waits), _MAX_WAITS_PER_DRAIN):
                extra = nc.sync.drain()
                extra.ins.sync_info = mybir.SyncInfo(
                    on_wait=waits[i : i + _MAX_WAITS_PER_DRAIN], on_update=[]
                )

        nc.all_engine_barrier()
        assert self.sems is not None
        popped = nc._tile_sem_poison_stack.pop()
        assert popped is self._sem_poison
        nc.clear_and_free_semaphores(list(self.sems.allocated().values()))
        nc.all_engine_barrier()


def build_nc(repeat: int = 0, rowtile_wx: bool = True):
    """repeat=0: straight-line kernel. repeat>=1: whole body wrapped in a
    For_i loop run `repeat` times (only used for timing measurements)."""
    import contextlib

    nc = bass.Bass()

    hT0 = nc.dram_tensor("hT0", [D, B], BF16, kind="ExternalInput")
    h80 = nc.dram_tensor("h80", [2 * 128, 2 * B], FP8, kind="ExternalInput")
    cT0 = nc.dram_tensor("cT0", [D, B], F32, kind="ExternalInput")
    wu = nc.dram_tensor("wu", [D, G], BF16, kind="ExternalInput")
    wuq = nc.dram_tensor("wuq", [128, 2 * 12 * 256], FP8, kind="ExternalInput")
    wx = nc.dram_tensor("wx", [4, G], BF16, kind="ExternalInput")
    w12 = nc.dram_tensor("w12", [D, 2], BF16, kind="ExternalInput")
    bvec = nc.dram_tensor("bvec", [G], F32, kind="ExternalInput")
    b12 = nc.dram_tensor("b12", [2, 1], F32, kind="ExternalInput")
    b2col = nc.dram_tensor("b2col", [K, 1], F32, kind="ExternalInput")
    x0 = nc.dram_tensor("x0", [1, B], BF16, kind="ExternalInput")
    ys1 = nc.dram_tensor("ys1", [K, B], F32, kind="ExternalOutput")
    ys2 = nc.dram_tensor("ys2", [K, B], F32, kind="ExternalOutput")

    with SplitDrainTileContext(nc) as tc:
        with contextlib.ExitStack() as ctx:
            wpool = ctx.enter_context(tc.tile_pool(name="w", bufs=1))
            hpool = ctx.enter_context(tc.tile_pool(name="h", bufs=16))
            h8pool = ctx.enter_context(tc.tile_pool(name="h8", bufs=8))
            cpool = ctx.enter_context(tc.tile_pool(name="c", bufs=16))
            gpool = ctx.enter_context(tc.tile_pool(name="g", bufs=16))
            tpool = ctx.enter_context(tc.tile_pool(name="t", bufs=4))
            xpool = ctx.enter_context(tc.tile_pool(name="x", bufs=3))
            ypool = ctx.enter_context(tc.tile_pool(name="y", bufs=4))
            opool = ctx.enter_context(tc.tile_pool(name="o", bufs=1))
            zps = ctx.enter_context(tc.tile_pool(name="zp", bufs=6, space="PSUM"))
            yps = ctx.enter_context(tc.tile_pool(name="yp", bufs=2, space="PSUM"))

            loop_cm = tc.For_i(0, repeat) if repeat else contextlib.nullcontext()
            with loop_cm:
                # --- weights + state init -------------------------------
                wu_sb = wpool.tile([128, KT * G], BF16, tag="wu")
                for k in range(KT):
                    nc.sync.dma_start(
                        wu_sb[:, k * G:(k + 1) * G], wu[k * 128:(k + 1) * 128, :]
                    )
                wuq_sb = wpool.tile([128, 2 * 12 * 256], FP8, tag="wuq")
                nc.sync.dma_start(wuq_sb[:, :], wuq[:, :])
                wx_sb = wpool.tile([128, G], BF16, tag="wx")
                w12_sb = wpool.tile([128, KT * 2], BF16, tag="w12")
                for k in range(KT):
                    nc.sync.dma_start(
                        w12_sb[:, 2 * k:2 * k + 2], w12[k * 128:(k + 1) * 128, :]
                    )
                b_sb = wpool.tile([128, MT], F32, tag="b")
                nc.sync.dma_start(b_sb[:, :], bvec[:].rearrange("(m p) -> p m", p=128))
                b12_sb = wpool.tile([2, 1], F32, tag="b12")
                nc.sync.dma_start(b12_sb[:, :], b12[:, :])
                b2c_sb = wpool.tile([K, 1], F32, tag="b2col")
                nc.sync.dma_start(b2c_sb[:, :], b2col[:, :])

                h_prev, c_prev = {}, {}
                for k in range(KT):
                    for n in range(NT):
                        ht = hpool.tile([128, N], BF16, tag="h")
                        nc.sync.dma_start(
                            ht[:, :], hT0[k * 128:(k + 1) * 128, n * N:(n + 1) * N]
                        )
                        h_prev[(k, n)] = ht
                        ct = cpool.tile([128, N], F32, tag="c")
                        nc.sync.dma_start(
                            ct[:, :], cT0[k * 128:(k + 1) * 128, n * N:(n + 1) * N]
                        )
                        c_prev[(k, n)] = ct
                h8_prev = {}
                for kt2 in range(2):
                    for n in range(NT):
                        h8t = h8pool.tile([128, 2, N], FP8, tag="h8",
                                          name=f"h8i_{kt2}_{n}")
                        nc.sync.dma_start(
                            h8t[:, :, :],
                            h80[kt2 * 128:(kt2 + 1) * 128, :]
                            .rearrange("p (two b) -> p two b", two=2)
                            [:, :, n * N:(n + 1) * N])
                        h8_prev[(kt2, n)] = h8t
                x_prev = xpool.tile([128, B], BF16, tag="x")
                nrows = 4 if rowtile_wx else 1
                for j in range(nrows):
                    nc.sync.dma_start(wx_sb[32 * j:32 * j + 1, :], wx[j:j + 1, :])
                    nc.sync.dma_start(x_prev[32 * j:32 * j + 1, :], x0[0:1, :])

                ys2pre = opool.tile([K, B], F32, tag="ys2pre")

                def dr_lhs(m, kt2):
                    # m: global gate tile 0..15 (i:0-3, f:4-7, o:12-15)
                    gi = {0: 0, 1: 1, 3: 2}[m // 4]
                    mi = gi * 4 + (m % 4)
                    off = (kt2 * 12 + mi) * 256
                    return wuq_sb[:, off:off + 256].rearrange(
                        "p (two m) -> p two m", two=2)

                # --- decode steps ---------------------------------------
                for t in range(K):
                    x_next = xpool.tile([128, B], BF16, tag="x")
                    h_new, c_new = {}, {}
                    h8_new = {}
                    for kt2 in range(2):
                        for n in range(NT):
                            h8_new[(kt2, n)] = h8pool.tile(
                                [128, 2, N], FP8, tag="h8",
                                name=f"h8n{t}_{kt2}_{n}")
                    for n in range(NT):
                        ns = slice(n * N, (n + 1) * N)
                        for k in range(KT):
                            gates = (8 + k, k, 4 + k, 12 + k)
                            zp_m = {}
                            drgates = []
                            for m in gates:
                                zp = zps.tile([128, N], F32, tag="z")
                                zp_m[m] = zp
                                if m // 4 == 2:   # g gate: bf16 path
                                    for kk in range(KT):
                                        nc.tensor.matmul(
                                            zp[:, :],
                                            wu_sb[:, kk * G + m * 128:kk * G + (m + 1) * 128],
                                            h_prev[(kk, n)][:, :],
                                            start=(kk == 0),
                                            stop=False,
                                        )
                                else:
                                    drgates.append(m)
                            # i,f,o fp8 DR, kt2-major: all pair-0 matmuls
                            # first so the tail-produced pair-1 h8 is needed
                            # as late as possible in the group
                            for kt2 in range(2):
                                for m in drgates:
                                    nc.tensor.matmul(
                                        zp_m[m][:, :],
                                        dr_lhs(m, kt2),
                                        h8_prev[(kt2, n)][:, :, :],
                                        start=(kt2 == 0),
                                        stop=False,
                                        perf_mode=DR,
                                    )
                            # rank-1 x closers, back-to-back on 4 row groups
                            for m in gates:
                                j = (m // 4) if rowtile_wx else 0
                                nc.tensor.matmul(
                                    zp_m[m][:, :],
                                    wx_sb[32 * j:32 * j + 1, m * 128:(m + 1) * 128],
                                    x_prev[32 * j:32 * j + 1, ns],
                                    start=False,
                                    stop=True,
                                    tile_position=(32 * j, 0) if rowtile_wx else None,
                                )
                            gt = {}
                            for m in gates:
                                g = gpool.tile([128, N], F32, tag="g")
                                func = AF.Tanh if m // 4 == 2 else AF.Sigmoid
                                nc.scalar.activation(
                                    g[:, :], zp_m[m][:, :], func, bias=b_sb[:, m:m + 1]
                                )
                                gt[m] = g
                            t1 = tpool.tile([128, N], F32, tag="t1")
                            nc.vector.tensor_mul(t1[:, :], gt[4 + k][:, :], c_prev[(k, n)][:, :])
                            t2 = tpool.tile([128, N], F32, tag="t2")
                            nc.vector.tensor_mul(t2[:, :], gt[k][:, :], gt[8 + k][:, :])
                            cn = cpool.tile([128, N], F32, tag="c")
                            nc.vector.tensor_add(cn[:, :], t1[:, :], t2[:, :])
                            tch = tpool.tile([128, N], F32, tag="tch")
                            nc.scalar.activation(tch[:, :], cn[:, :], AF.Tanh)
                            hn = hpool.tile([128, N], BF16, tag="h")
                            nc.vector.tensor_mul(hn[:, :], gt[12 + k][:, :], tch[:, :])
                            nc.vector.tensor_copy(
                                h8_new[(k // 2, n)][:, k % 2, :], hn[:, :])
                            h_new[(k, n)] = hn
                            c_new[(k, n)] = cn
                        # y head for chunk n
                        yp = yps.tile([2, N], F32, tag="y")
                        for k in range(KT):
                            nc.tensor.matmul(
                                yp[:, :],
                                w12_sb[:, 2 * k:2 * k + 2],
                                h_new[(k, n)][:, :],
                                start=(k == 0),
                                stop=(k == KT - 1),
                            )
                        yr1 = ypool.tile([2, N], F32, tag="yr1")
                        nc.scalar.activation(
                            yr1[0:1, :], yp[0:1, :], AF.Sigmoid, bias=b12_sb[0:1, 0:1]
                        )
                        nc.sync.dma_start(ys1[t:t + 1, ns], yr1[0:1, :])
                        yr2 = ypool.tile([2, N], F32, tag="yr2")
                        # x feedback and the y2 row copy ride VectorE to keep
                        # ScalarE (the second-busiest engine) lean
                        nc.vector.tensor_copy(x_next[0:1, ns], yr1[0:1, :])
                        nc.vector.tensor_copy(yr2[0:2, :], yp[0:2, :])
                        nc.sync.dma_start(ys2pre[t:t + 1, ns], yr2[1:2, :])
                    if rowtile_wx:
                        for j in range(1, 4):
                            nc.sync.dma_start(
                                x_next[32 * j:32 * j + 1, :], x_next[0:1, :]
                            )
                    h_prev, c_prev, x_prev = h_new, c_new, x_next
                    h8_prev = h8_new

                # --- batched elu tail: y2 = relu(p) + exp(min(p,0)) - 1 --
                pb = opool.tile([K, B], F32, tag="elu_p")
                nc.scalar.activation(
                    pb[:, :], ys2pre[:, :], AF.Identity, bias=b2c_sb[:, 0:1]
                )
                r = opool.tile([K, B], F32, tag="elu_r")
                nc.scalar.activation(r[:, :], pb[:, :], AF.Relu)
                neg = opool.tile([K, B], F32, tag="elu_n")
                nc.vector.tensor_sub(neg[:, :], pb[:, :], r[:, :])
                e = opool.tile([K, B], F32, tag="elu_e")
                nc.scalar.activation(e[:, :], neg[:, :], AF.Exp)
                s = opool.tile([K, B], F32, tag="elu_s")
                nc.vector.tensor_add(s[:, :], r[:, :], e[:, :])
                y2f = opool.tile([K, B], F32, tag="elu_y")
                nc.vector.tensor_scalar_add(y2f[:, :], s[:, :], -1.0)
                nc.sync.dma_start(ys2[:, :], y2f[:, :])

    _split_waits(nc)
    return nc


def make_in_map(initial, encoder_hidden, encoder_cell, Wx, Wu, b, w1, b1, w2, b2):
    """Per-core input dict from this core's batch shard (numpy fp32 arrays)."""
    import ml_dtypes
    E4 = ml_dtypes.float8_e4m3
    bf = lambda a: np.ascontiguousarray(a).astype(ml_dtypes.bfloat16)
    f32 = lambda a: np.ascontiguousarray(a, dtype=np.float32)

    h0T = np.ascontiguousarray(encoder_hidden, dtype=np.float32).T
    h80 = np.zeros((256, 2 * B), dtype=E4)
    for kt2 in range(2):
        for i in range(2):
            h80[kt2 * 128:(kt2 + 1) * 128, i * B:(i + 1) * B] = (
                h0T[256 * kt2 + 128 * i:256 * kt2 + 128 * (i + 1), :].astype(E4))

    Wu = np.asarray(Wu, np.float32)
    colblk = {"i": 0, "f": 1, "o": 3}
    wuq = np.zeros((128, 2 * 12 * 256), dtype=E4)
    for kt2 in range(2):
        for gi, gname in enumerate(("i", "f", "o")):
            for kt in range(4):
                mi = gi * 4 + kt
                off = (kt2 * 12 + mi) * 256
                col = colblk[gname] * D + kt * 128
                for i in range(2):
                    blk = Wu[256 * kt2 + 128 * i:256 * kt2 + 128 * (i + 1),
                             col:col + 128]
                    wuq[:, off + i * 128:off + (i + 1) * 128] = blk.astype(E4)

    return {
        "wuq": wuq,
        "h80": h80,
        "hT0": bf(encoder_hidden.T),
        "cT0": f32(encoder_cell.T),
        "wu": bf(Wu),
        "wx": bf(np.broadcast_to(Wx, (4, G))),
        "w12": bf(np.concatenate([w1, w2], axis=1)),
        "bvec": f32(b),
        "b12": np.array([[np.float32(b1[0])], [np.float32(b2[0])]], dtype=np.float32),
        "b2col": np.full((K, 1), np.float32(b2[0]), dtype=np.float32),
        "x0": bf(initial[:, 0, :].T),
    }


_CACHE = {}


def _get_nc():
    if "nc" not in _CACHE:
        _CACHE["nc"] = build_nc(repeat=0, rowtile_wx=True)
    return _CACHE["nc"]


def kernel(initial, encoder_hidden, encoder_cell, Wx, Wu, b, w1, b1, w2, b2):
    from concourse import bass_utils

    initial = np.asarray(initial, dtype=np.float32)
    encoder_hidden = np.asarray(encoder_hidden, dtype=np.float32)
    encoder_cell = np.asarray(encoder_cell, dtype=np.float32)
    Wx = np.asarray(Wx, dtype=np.float32)
    Wu = np.asarray(Wu, dtype=np.float32)
    b = np.asarray(b, dtype=np.float32)
    w1 = np.asarray(w1, dtype=np.float32)
    b1 = np.asarray(b1, dtype=np.float32)
    w2 = np.asarray(w2, dtype=np.float32)
    b2 = np.asarray(b2, dtype=np.float32)

    nc = _get_nc()
    in_maps = []
    for c in range(NCORES):
        sl = slice(c * B, (c + 1) * B)
        in_maps.append(
            make_in_map(initial[sl], encoder_hidden[sl], encoder_cell[sl],
                        Wx, Wu, b, w1, b1, w2, b2)
        )
    res = bass_utils.run_bass_kernel_spmd(nc, in_maps, core_ids=list(range(NCORES)))
    out1 = np.concatenate([res.results[c]["ys1"].T for c in range(NCORES)], axis=0)
    out2 = np.concatenate([res.results[c]["ys2"].T for c in range(NCORES)], axis=0)
    return (np.ascontiguousarray(out1, dtype=np.float32),
            np.ascontiguousarray(out2, dtype=np.float32))



# revision 3
# speedup vs baseline: 1.5561x; 1.5561x over previous
"""Trainium2 Bass kernel for the autoregressive LSTM decoder problem.

v4: engine-balance redesign. The v3 bottleneck was the Activation engine
(620us busy of 833us sim: 40 acts/step at [128,512]); v4 cuts Act work and
moves everything movable off it:
  * g-gate tanh via the sigmoid identity tanh(z) = 2*sigmoid(2z)-1 (Wg, bg
    pre-scaled by 2 host-side), so all four gates are Sigmoid and pairs of
    gates sharing a 2-bank PSUM tile evacuate in ONE wide activation
    ([128,2,512]): 2 acts/group instead of 4.
  * biases folded into the rank-2 x-closers (lhsT rows = [Wx_m; b_m], rhs
    rows = [x; ones]) so wide acts need no per-partition bias.
  * elementwise chain in fp16/bf16 (DVE 2x perf mode): c is fp16, gate
    tiles fp16, h bf16. u = 2*sig_g-1 runs on the idle GPSIMD/Pool engine.
  * tanh(c), h-mul, and h8 copy for group G are emitted during group G+1 so
    the in-order Act/DVE queues never head-of-line block on the c chain.
  * x feedback: y1 sigmoid writes the x row directly (bf16, also DMA'd out
    as ys1), 3 small per-chunk DMAs fan it out to partitions 32/64/96;
    'ones' rows at partitions 1/33/65/97 are persistent. No cross-chunk
    join, nothing on the step-boundary critical path.
  * PSUM: zif (i,f) 2-bank tile bufs=1, zog (o,g) 2-bank bufs=2, y 2x1
    banks = 8 banks exactly; if-phase first so the zif round-trip hides
    under the og-phase PE time.

Full-input contract: kernel(**inputs) takes the unsharded numpy inputs
(B=8192, D=512, K=24) and returns (out1, out2), each [B, K] float32.
Data-parallel over 8 NeuronCores, B/8 = 1024 batch per core; state is kept
transposed on-chip (h,c as [D, B_shard]) so the per-step gate matmul lands
in PSUM gate-major with no transposes in the decode loop. i,f,o gate
matmuls are fp8e4 DoubleRow (2x PE), g stays bf16.
"""

import sys

import numpy as np

for _p in ("/opt/trn_rl_repo", "/root/.axon_site/_ro/trn_rl_repo"):
    if _p not in sys.path:
        sys.path.append(_p)

import concourse.bass as bass
import concourse.mybir as mybir
from concourse.tile import TileContext
from concourse.vector_clock import ScopedClock

F32 = mybir.dt.float32
BF16 = mybir.dt.bfloat16
FP16 = mybir.dt.float16
FP8 = mybir.dt.float8e4
DR = mybir.MatmulPerfMode.DoubleRow
AF = mybir.ActivationFunctionType
ALU = mybir.AluOpType

D = 512
B = 1024          # batch per core
NCORES = 8
K = 24
G = 4 * D         # 2048 gate rows
KT = D // 128     # 4 k tiles
NT = B // 512     # 2 batch chunks
N = 512

_MAX_WAITS_PER_DRAIN = 1


def _split_waits(nc):
    """The walrus build in this container accepts at most one semaphore wait
    per instruction. Rebuild every basic block, hoisting all-but-one wait of
    any overloaded instruction onto same-engine InstEventSemaphore
    instructions inserted immediately before it — the engine blocks at the
    same program point for the same conditions, so this is
    semantics-preserving."""
    n_new = 0
    for f in nc.m.functions:
        for blk in f.blocks:
            insts = list(blk.instructions)
            out = []
            changed = False
            for inst in insts:
                si = inst.sync_info
                waits = list(si.on_wait) if si is not None else []
                if len(waits) > 1:
                    changed = True
                    excess, keep = waits[:-1], waits[-1:]
                    for w in excess:
                        ev = mybir.InstEventSemaphore(
                            name=f"splitw-{n_new}", ins=[], outs=[],
                            engine=inst.engine,
                        )
                        ev.sync_info = mybir.SyncInfo(on_wait=[w], on_update=[])
                        nc.register_instruction(ev, overwrite=True)
                        out.append(ev)
                        n_new += 1
                    inst.sync_info = mybir.SyncInfo(
                        on_wait=keep, on_update=list(si.on_update)
                    )
                out.append(inst)
            if changed:
                blk.instructions = out
    return n_new


class SplitDrainTileContext(TileContext):
    """The walrus build in this container rejects Drain (CTRL_NO)
    instructions carrying more than ~2 sync waits; split the tail drain's
    waits across a chain of Drain instructions, one wait each."""

    def _drain_and_barrier(self, tick_clock, wait_clock):
        nc = self.nc
        drain_inst = nc.sync.drain()
        wait_clock.add_sem_waits(
            drain_inst.ins, ScopedClock({None: tick_clock.global_clock})
        )
        si = drain_inst.ins.sync_info
        waits = list(si.on_wait) if si is not None else []
        if len(waits) > _MAX_WAITS_PER_DRAIN:
            drain_inst.ins.sync_info = mybir.SyncInfo(
                on_wait=waits[:_MAX_WAITS_PER_DRAIN], on_update=[]
            )
            for i in range(_MAX_WAITS_PER_DRAIN, len(waits), _MAX_WAITS_PER_DRAIN):
                extra = nc.sync.drain()
                extra.ins.sync_info = mybir.SyncInfo(
                    on_wait=waits[i : i + _MAX_WAITS_PER_DRAIN], on_update=[]
                )

        nc.all_engine_barrier()
        assert self.sems is not None
        popped = nc._tile_sem_poison_stack.pop()
        assert popped is self._sem_poison
        nc.clear_and_free_semaphores(list(self.sems.allocated().values()))
        nc.all_engine_barrier()


def build_nc(repeat: int = 0, rowtile_wx: bool = True):
    """repeat=0: straight-line kernel. repeat>=1: whole body wrapped in a
    For_i loop run `repeat` times (only used for timing measurements)."""
    import contextlib

    nc = bass.Bass()

    hT0 = nc.dram_tensor("hT0", [D, B], BF16, kind="ExternalInput")
    h80 = nc.dram_tensor("h80", [2 * 128, 2 * B], FP8, kind="ExternalInput")
    cT0 = nc.dram_tensor("cT0", [D, B], FP16, kind="ExternalInput")
    wu = nc.dram_tensor("wu", [D, G], BF16, kind="ExternalInput")
    wuq = nc.dram_tensor("wuq", [128, 2 * 12 * 256], FP8, kind="ExternalInput")
    wxb = nc.dram_tensor("wxb", [128, G], BF16, kind="ExternalInput")
    w12 = nc.dram_tensor("w12", [D, 2], BF16, kind="ExternalInput")
    b12 = nc.dram_tensor("b12", [2, 1], F32, kind="ExternalInput")
    b2col = nc.dram_tensor("b2col", [K, 1], F32, kind="ExternalInput")
    xinit = nc.dram_tensor("xinit", [128, B], BF16, kind="ExternalInput")
    ys1 = nc.dram_tensor("ys1", [K, B], BF16, kind="ExternalOutput")
    ys2 = nc.dram_tensor("ys2", [K, B], F32, kind="ExternalOutput")

    with SplitDrainTileContext(nc) as tc:
        with contextlib.ExitStack() as ctx:
            wpool = ctx.enter_context(tc.tile_pool(name="w", bufs=1))
            hpool = ctx.enter_context(tc.tile_pool(name="h", bufs=16))
            h8pool = ctx.enter_context(tc.tile_pool(name="h8", bufs=8))
            cpool = ctx.enter_context(tc.tile_pool(name="c", bufs=16))
            gpool = ctx.enter_context(tc.tile_pool(name="g", bufs=8))
            tpool = ctx.enter_context(tc.tile_pool(name="t", bufs=6))
            ypool = ctx.enter_context(tc.tile_pool(name="y", bufs=4))
            opool = ctx.enter_context(tc.tile_pool(name="o", bufs=1))
            zifps = ctx.enter_context(tc.tile_pool(name="zif", bufs=1, space="PSUM"))
            zogps = ctx.enter_context(tc.tile_pool(name="zog", bufs=2, space="PSUM"))
            yps = ctx.enter_context(tc.tile_pool(name="yp", bufs=2, space="PSUM"))

            loop_cm = tc.For_i(0, repeat) if repeat else contextlib.nullcontext()
            with loop_cm:
                # --- weights + state init -------------------------------
                wu_sb = wpool.tile([128, KT * G], BF16, tag="wu")
                for k in range(KT):
                    nc.sync.dma_start(
                        wu_sb[:, k * G:(k + 1) * G], wu[k * 128:(k + 1) * 128, :]
                    )
                wuq_sb = wpool.tile([128, 2 * 12 * 256], FP8, tag="wuq")
                nc.sync.dma_start(wuq_sb[:, :], wuq[:, :])
                wxb_sb = wpool.tile([128, G], BF16, tag="wxb")
                nc.sync.dma_start(wxb_sb[:, :], wxb[:, :])
                w12_sb = wpool.tile([128, KT * 2], BF16, tag="w12")
                for k in range(KT):
                    nc.sync.dma_start(
                        w12_sb[:, 2 * k:2 * k + 2], w12[k * 128:(k + 1) * 128, :]
                    )
                b12_sb = wpool.tile([2, 1], F32, tag="b12")
                nc.sync.dma_start(b12_sb[:, :], b12[:, :])
                b2c_sb = wpool.tile([K, 1], F32, tag="b2col")
                nc.sync.dma_start(b2c_sb[:, :], b2col[:, :])

                # persistent x tiles (double-buffered across steps); rows
                # 0/32/64/96 hold x, rows 1/33/65/97 hold the ones vector
                # for the bias closer (never rewritten after init).
                x_ab = []
                for s in range(2):
                    xt = wpool.tile([128, B], BF16, tag=f"x{s}", name=f"x{s}")
                    nc.sync.dma_start(xt[:, :], xinit[:, :])
                    x_ab.append(xt)

                h_prev, c_prev = {}, {}
                for k in range(KT):
                    for n in range(NT):
                        ht = hpool.tile([128, N], BF16, tag="h")
                        nc.sync.dma_start(
                            ht[:, :], hT0[k * 128:(k + 1) * 128, n * N:(n + 1) * N]
                        )
                        h_prev[(k, n)] = ht
                        ct = cpool.tile([128, N], FP16, tag="c")
                        nc.sync.dma_start(
                            ct[:, :], cT0[k * 128:(k + 1) * 128, n * N:(n + 1) * N]
                        )
                        c_prev[(k, n)] = ct
                h8_prev = {}
                for kt2 in range(2):
                    for n in range(NT):
                        h8t = h8pool.tile([128, 2, N], FP8, tag="h8",
                                          name=f"h8i_{kt2}_{n}")
                        nc.sync.dma_start(
                            h8t[:, :, :],
                            h80[kt2 * 128:(kt2 + 1) * 128, :]
                            .rearrange("p (two b) -> p two b", two=2)
                            [:, :, n * N:(n + 1) * N])
                        h8_prev[(kt2, n)] = h8t

                ys2pre = opool.tile([K, B], F32, tag="ys2pre")

                def dr_lhs(m, kt2):
                    # m: global gate tile 0..15 (i:0-3, f:4-7, o:12-15)
                    gi = {0: 0, 1: 1, 3: 2}[m // 4]
                    mi = gi * 4 + (m % 4)
                    off = (kt2 * 12 + mi) * 256
                    return wuq_sb[:, off:off + 256].rearrange(
                        "p (two m) -> p two m", two=2)

                # --- decode steps ---------------------------------------
                for t in range(K):
                    x_cur = x_ab[t % 2]
                    x_next = x_ab[(t + 1) % 2]
                    h_new, c_new = {}, {}
                    h8_new = {}
                    for kt2 in range(2):
                        for n in range(NT):
                            h8_new[(kt2, n)] = h8pool.tile(
                                [128, 2, N], FP8, tag="h8",
                                name=f"h8n{t}_{kt2}_{n}")

                    # deferred-tail state: (n, k, cn, o_ap) of the previous
                    # group, flushed one group later to keep Act/DVE queues
                    # from head-of-line blocking on the c chain.
                    pending = None

                    def flush_pending():
                        nonlocal pending
                        if pending is None:
                            return
                        pn, pk, pcn, pgog = pending
                        pending = None
                        tch = tpool.tile([128, N], FP16, tag="tch")
                        nc.scalar.activation(tch[:, :], pcn[:, :], AF.Tanh)
                        hn = hpool.tile([128, N], BF16, tag="h")
                        nc.vector.tensor_mul(hn[:, :], pgog[:, 0, :], tch[:, :])
                        nc.vector.tensor_copy(
                            h8_new[(pk // 2, pn)][:, pk % 2, :], hn[:, :])
                        h_new[(pk, pn)] = hn
                        if pk == KT - 1:
                            emit_yhead(pn)

                    def emit_yhead(n):
                        ns = slice(n * N, (n + 1) * N)
                        yp = yps.tile([2, N], F32, tag="y")
                        for kk in range(KT):
                            nc.tensor.matmul(
                                yp[:, :],
                                w12_sb[:, 2 * kk:2 * kk + 2],
                                h_new[(kk, n)][:, :],
                                start=(kk == 0),
                                stop=(kk == KT - 1),
                            )
                        # y1 sigmoid writes next-step x row directly (bf16);
                        # the same row is DMA'd out as ys1 and fanned out to
                        # partitions 32/64/96 for the rank-2 closers.
                        nc.scalar.activation(
                            x_next[0:1, ns], yp[0:1, :], AF.Sigmoid,
                            bias=b12_sb[0:1, 0:1]
                        )
                        nc.sync.dma_start(ys1[t:t + 1, ns], x_next[0:1, ns])
                        for j in range(1, 4):
                            nc.sync.dma_start(
                                x_next[32 * j:32 * j + 1, ns], x_next[0:1, ns]
                            )
                        yr2 = ypool.tile([2, N], F32, tag="yr2")
                        nc.vector.tensor_copy(yr2[0:2, :], yp[0:2, :])
                        nc.sync.dma_start(ys2pre[t:t + 1, ns], yr2[1:2, :])

                    def closer(zp_slice, m, ns):
                        j = m // 4
                        r = 32 * j
                        nc.tensor.matmul(
                            zp_slice,
                            wxb_sb[r:r + 2, m * 128:(m + 1) * 128],
                            x_cur[r:r + 2, ns],
                            start=False,
                            stop=True,
                            tile_position=(r, 0),
                        )

                    for n in range(NT):
                        ns = slice(n * N, (n + 1) * N)
                        for k in range(KT):
                            mi, mf, mg, mo = k, 4 + k, 8 + k, 12 + k
                            # --- if-phase: i,f fp8 DR + closers + wide sig
                            zif = zifps.tile([128, 2, N], F32, tag="zif")
                            for kt2 in range(2):
                                for sl, m in ((0, mi), (1, mf)):
                                    nc.tensor.matmul(
                                        zif[:, sl, :],
                                        dr_lhs(m, kt2),
                                        h8_prev[(kt2, n)][:, :, :],
                                        start=(kt2 == 0),
                                        stop=False,
                                        perf_mode=DR,
                                    )
                            closer(zif[:, 0, :], mi, ns)
                            closer(zif[:, 1, :], mf, ns)
                            gif = gpool.tile([128, 2, N], FP16, tag="gif")
                            nc.scalar.activation(
                                gif[:, :, :], zif[:, :, :], AF.Sigmoid
                            )
                            # --- og-phase: o fp8 DR, g bf16 (2x-scaled
                            # weights; tanh via 2*sigmoid-1) + closers
                            zog = zogps.tile([128, 2, N], F32, tag="zog")
                            for kt2 in range(2):
                                nc.tensor.matmul(
                                    zog[:, 0, :],
                                    dr_lhs(mo, kt2),
                                    h8_prev[(kt2, n)][:, :, :],
                                    start=(kt2 == 0),
                                    stop=False,
                                    perf_mode=DR,
                                )
                            for kk in range(KT):
                                nc.tensor.matmul(
                                    zog[:, 1, :],
                                    wu_sb[:, kk * G + mg * 128:kk * G + (mg + 1) * 128],
                                    h_prev[(kk, n)][:, :],
                                    start=(kk == 0),
                                    stop=False,
                                )
                            closer(zog[:, 0, :], mo, ns)
                            closer(zog[:, 1, :], mg, ns)
                            gog = gpool.tile([128, 2, N], FP16, tag="gog")
                            nc.scalar.activation(
                                gog[:, :, :], zog[:, :, :], AF.Sigmoid
                            )
                            # --- deferred tail of the previous group
                            flush_pending()
                            # --- c update for this group
                            u = tpool.tile([128, N], FP16, tag="u")
                            nc.gpsimd.tensor_scalar(
                                u[:, :], gog[:, 1, :], 2.0, -1.0,
                                ALU.mult, ALU.add)
                            t1 = tpool.tile([128, N], FP16, tag="t1")
                            nc.vector.tensor_mul(
                                t1[:, :], gif[:, 1, :], c_prev[(k, n)][:, :])
                            t2 = tpool.tile([128, N], FP16, tag="t2")
                            nc.vector.tensor_mul(t2[:, :], gif[:, 0, :], u[:, :])
                            cn = cpool.tile([128, N], FP16, tag="c")
                            nc.vector.tensor_add(cn[:, :], t1[:, :], t2[:, :])
                            c_new[(k, n)] = cn
                            pending = (n, k, cn, gog)
                    flush_pending()
                    h_prev, c_prev = h_new, c_new
                    h8_prev = h8_new

                # --- batched elu tail: y2 = relu(p) + exp(min(p,0)) - 1 --
                pb = opool.tile([K, B], F32, tag="elu_p")
                nc.scalar.activation(
                    pb[:, :], ys2pre[:, :], AF.Identity, bias=b2c_sb[:, 0:1]
                )
                r = opool.tile([K, B], F32, tag="elu_r")
                nc.scalar.activation(r[:, :], pb[:, :], AF.Relu)
                neg = opool.tile([K, B], F32, tag="elu_n")
                nc.vector.tensor_sub(neg[:, :], pb[:, :], r[:, :])
                e = opool.tile([K, B], F32, tag="elu_e")
                nc.scalar.activation(e[:, :], neg[:, :], AF.Exp)
                s = opool.tile([K, B], F32, tag="elu_s")
                nc.vector.tensor_add(s[:, :], r[:, :], e[:, :])
                y2f = opool.tile([K, B], F32, tag="elu_y")
                nc.vector.tensor_scalar_add(y2f[:, :], s[:, :], -1.0)
                nc.sync.dma_start(ys2[:, :], y2f[:, :])

    _split_waits(nc)
    return nc


def make_in_map(initial, encoder_hidden, encoder_cell, Wx, Wu, b, w1, b1, w2, b2):
    """Per-core input dict from this core's batch shard (numpy fp32 arrays)."""
    import ml_dtypes
    E4 = ml_dtypes.float8_e4m3
    bf = lambda a: np.ascontiguousarray(a).astype(ml_dtypes.bfloat16)
    f32 = lambda a: np.ascontiguousarray(a, dtype=np.float32)

    h0T = np.ascontiguousarray(encoder_hidden, dtype=np.float32).T
    h80 = np.zeros((256, 2 * B), dtype=E4)
    for kt2 in range(2):
        for i in range(2):
            h80[kt2 * 128:(kt2 + 1) * 128, i * B:(i + 1) * B] = (
                h0T[256 * kt2 + 128 * i:256 * kt2 + 128 * (i + 1), :].astype(E4))

    Wu = np.asarray(Wu, np.float32)
    colblk = {"i": 0, "f": 1, "o": 3}
    wuq = np.zeros((128, 2 * 12 * 256), dtype=E4)
    for kt2 in range(2):
        for gi, gname in enumerate(("i", "f", "o")):
            for kt in range(4):
                mi = gi * 4 + kt
                off = (kt2 * 12 + mi) * 256
                col = colblk[gname] * D + kt * 128
                for i in range(2):
                    blk = Wu[256 * kt2 + 128 * i:256 * kt2 + 128 * (i + 1),
                             col:col + 128]
                    wuq[:, off + i * 128:off + (i + 1) * 128] = blk.astype(E4)

    # g-block (cols [2D,3D)) scaled by 2: tanh(z) = 2*sigmoid(2z) - 1.
    Wu2 = Wu.copy()
    Wu2[:, 2 * D:3 * D] *= 2.0
    Wx2 = np.asarray(Wx, np.float32).reshape(1, G).copy()
    Wx2[0, 2 * D:3 * D] *= 2.0
    b2x = np.asarray(b, np.float32).copy()
    b2x[2 * D:3 * D] *= 2.0

    # wxb rows 32j = Wx (full G cols), rows 32j+1 = b; j = gate class.
    wxb = np.zeros((128, G), dtype=np.float32)
    for j in range(4):
        wxb[32 * j, :] = Wx2[0, :]
        wxb[32 * j + 1, :] = b2x

    # xinit: x0 at rows 0/32/64/96, ones at rows 1/33/65/97.
    x0row = np.asarray(initial, np.float32)[:, 0, 0]
    xinit = np.zeros((128, B), dtype=np.float32)
    for j in range(4):
        xinit[32 * j, :] = x0row
        xinit[32 * j + 1, :] = 1.0

    return {
        "wuq": wuq,
        "h80": h80,
        "hT0": bf(encoder_hidden.T),
        "cT0": np.ascontiguousarray(encoder_cell.T).astype(np.float16),
        "wu": bf(Wu2),
        "wxb": bf(wxb),
        "w12": bf(np.concatenate([w1, w2], axis=1)),
        "b12": np.array([[np.float32(b1[0])], [np.float32(b2[0])]], dtype=np.float32),
        "b2col": np.full((K, 1), np.float32(b2[0]), dtype=np.float32),
        "xinit": bf(xinit),
    }


_CACHE = {}


def _get_nc():
    if "nc" not in _CACHE:
        _CACHE["nc"] = build_nc(repeat=0)
    return _CACHE["nc"]


def kernel(initial, encoder_hidden, encoder_cell, Wx, Wu, b, w1, b1, w2, b2):
    from concourse import bass_utils

    initial = np.asarray(initial, dtype=np.float32)
    encoder_hidden = np.asarray(encoder_hidden, dtype=np.float32)
    encoder_cell = np.asarray(encoder_cell, dtype=np.float32)
    Wx = np.asarray(Wx, dtype=np.float32)
    Wu = np.asarray(Wu, dtype=np.float32)
    b = np.asarray(b, dtype=np.float32)
    w1 = np.asarray(w1, dtype=np.float32)
    b1 = np.asarray(b1, dtype=np.float32)
    w2 = np.asarray(w2, dtype=np.float32)
    b2 = np.asarray(b2, dtype=np.float32)

    nc = _get_nc()
    in_maps = []
    for c in range(NCORES):
        sl = slice(c * B, (c + 1) * B)
        in_maps.append(
            make_in_map(initial[sl], encoder_hidden[sl], encoder_cell[sl],
                        Wx, Wu, b, w1, b1, w2, b2)
        )
    res = bass_utils.run_bass_kernel_spmd(nc, in_maps, core_ids=list(range(NCORES)))
    out1 = np.concatenate(
        [res.results[c]["ys1"].astype(np.float32).T for c in range(NCORES)], axis=0)
    out2 = np.concatenate([res.results[c]["ys2"].T for c in range(NCORES)], axis=0)
    return (np.ascontiguousarray(out1, dtype=np.float32),
            np.ascontiguousarray(out2, dtype=np.float32))
